# revision 8
# baseline (speedup 1.0000x reference)
import numpy as np

EPS = 1e-5

# nn_AssociativeAttention: B=2, L=2048, D=256, H=8, h=32.
# 8 cores x 2 (b,h) lanes each. bf16 matmul pipeline, v3:
#  A: fused {q,v'} {k',gp} projections [d,l]; same-base products; colsum
#     matmuls -> sim/|k|^2/|v|^2/glog per position (stacked [64,BS] psum)
#  B: cumsum-only scan chain in [32,128] (max-scan dropped; s_scan>=1),
#     w broadcast via PE ones-matmul -> qw = q*w
#  C: chunked causal linear attention in transposed-ct form:
#     ctT[e,j] = sum_i khR[i,e] atm[i,j] + sum_d S[d,e] qw[d,j]
#     with atm = mask * P', P' = v'^T qw; ghat folded into khR drains;
#     dS accumulated in PSUM. Partial out-proj y (bf16) per core.
# Host reduces 4 cores per batch and adds wo_b.

B, L, D, H, HD = 2, 2048, 256, 8, 32
N_CORES = 8
C = 128
NCH = L // C
SEG = 16
NBLK = 4
BS = L // NBLK  # 512


def _build_bass():
    import os as _os
    DEBUG = int(_os.environ.get('KV2_DEBUG', '0'))
    SAFES = int(_os.environ.get('KV2_SAFES', '0'))
    import sys
    if '/opt/trn_rl_repo' not in sys.path:
        sys.path.insert(0, '/opt/trn_rl_repo')
    from contextlib import ExitStack
    import concourse.bacc as bacc
    import concourse.tile as tile
    from concourse import mybir

    f32 = mybir.dt.float32
    bf16 = mybir.dt.bfloat16
    AF = mybir.ActivationFunctionType
    OP = mybir.AluOpType

    nc = bacc.Bacc()
    dt = nc.dram_tensor
    xT_d = dt("xT", [128, 2, L], bf16, kind="ExternalInput")
    # blob cols: w1T(0:256) w2T(256:512) cw1(512:544) cw2(544:576)
    #   woT(576:832, rows 0:64) identb(832:960) mask2(960:1216, 2x128)
    #   ones2(1216:1280, rows 0:2)
    wb_d = dt("wb", [128, 1280], bf16, kind="ExternalInput")
    sb_d = dt("sb", [32, 34], f32, kind="ExternalInput")  # qs|gb|lt32
    y_d = dt("y", [L, D], bf16, kind="ExternalOutput")
    if DEBUG:
        dqk_d = dt("dqk", [128, L], bf16, kind="ExternalOutput")
        dvgp_d = dt("dvgp", [128, L], bf16, kind="ExternalOutput")
        dsc_d = dt("dsc", [32, 128, 6], f32, kind="ExternalOutput")
        dkh_d = dt("dkh", [128, NCH * 64], bf16, kind="ExternalOutput")
        dqw_d = dt("dqw", [64, L], bf16, kind="ExternalOutput")

    with tile.TileContext(nc) as tc:
        with ExitStack() as ctx:
            const = ctx.enter_context(tc.tile_pool(name="const", bufs=1))
            big = ctx.enter_context(tc.tile_pool(name="big", bufs=1))
            work = ctx.enter_context(tc.tile_pool(name="work", bufs=2))
            outp = ctx.enter_context(tc.tile_pool(name="outp", bufs=3))

            wb = const.tile([128, 1280], bf16)
            nc.sync.dma_start(wb[:], wb_d[:, :])
            sb = const.tile([32, 34], f32)
            nc.sync.dma_start(sb[:], sb_d[:, :])
            w1T = wb[:, 0:256].rearrange("p (f m) -> p f m", f=2)
            w2T = wb[:, 256:512].rearrange("p (f m) -> p f m", f=2)
            cw1 = wb[:, 512:544]
            cw2 = wb[:, 544:576]
            woT = wb[:, 576:832]
            identb = wb[:, 832:960]
            mask2 = wb[:, 960:1216].rearrange("p (a t) -> p a t", a=2)
            ones2 = wb[0:2, 1216:1280]
            qs32 = sb[:, 0:1]
            gb = sb[:, 1:2]
            lt32 = sb[:, 2:34]

            xT = big.tile([128, 2, L], bf16)
            for blk in range(NBLK):
                bsl = slice(blk * BS, (blk + 1) * BS)
                nc.sync.dma_start(xT[:, :, bsl], xT_d[:, :, bsl])

            z2 = const.tile([32, 2, 128], f32)
            nc.vector.memset(z2[:], 0.0)

            # ---- phase A ----
            qk = big.tile([128, L], bf16)    # q 0:64, v' 64:128
            vgp = big.tile([128, L], bf16)   # k' 0:64, gp 64:128
            v0 = big.tile([64, L], bf16)     # v' shifted to base 0
            sg4 = big.tile([64, L], f32)     # rows 0:4 sim/nk, 32:36 glog/nv
            with tc.tile_pool(name="pmm", bufs=2, space="PSUM") as pmm, \
                 tc.tile_pool(name="psmall", bufs=2, space="PSUM") as psmall:
                for blk in range(NBLK):
                    bsl = slice(blk * BS, (blk + 1) * BS)
                    ps1 = pmm.tile([128, BS], f32, tag="mm")
                    for f in range(2):
                        nc.tensor.matmul(ps1[:], w1T[:, f, :], xT[:, f, bsl],
                                         start=(f == 0), stop=(f == 1))
                    nc.scalar.copy(qk[:, bsl], ps1[:])
                    ps2 = pmm.tile([128, BS], f32, tag="mm")
                    for f in range(2):
                        nc.tensor.matmul(ps2[:], w2T[:, f, :], xT[:, f, bsl],
                                         start=(f == 0), stop=(f == 1))
                    nc.vector.tensor_copy(vgp[:, bsl], ps2[:])
                    s1 = work.tile([128, BS], bf16, tag="s1")
                    nc.vector.tensor_mul(s1[0:64, :], qk[0:64, bsl], vgp[0:64, bsl])
                    nc.gpsimd.tensor_mul(s1[64:128, :], vgp[0:64, bsl], vgp[0:64, bsl])
                    s2 = work.tile([128, BS], bf16, tag="s2")
                    nc.gpsimd.tensor_mul(s2[0:64, :], qk[64:128, bsl], qk[64:128, bsl])
                    nc.vector.tensor_mul(s2[64:128, :], qk[64:128, bsl], vgp[64:128, bsl])
                    cs = psmall.tile([64, BS], f32, tag="cs")
                    hpA = tc.high_priority()
                    hpA.__enter__()
                    nc.tensor.matmul(cs[0:32, :], cw1[:, :], s1[:], start=True, stop=True)
                    nc.tensor.matmul(cs[32:64, :], cw2[:, :], s2[:], start=True, stop=True)
                    if blk % 2 == 0:
                        nc.scalar.copy(sg4[:, bsl], cs[:])
                    else:
                        nc.vector.tensor_copy(sg4[:, bsl], cs[:])
                    hpA.__exit__(None, None, None)
            nc.sync.dma_start(v0[:, :], qk[64:128, :])

            # ---- phase B ----
            po_cm = tc.tile_pool(name="po", bufs=5, space="PSUM")
            po = po_cm.__enter__()
            pS_cm = tc.tile_pool(name="pS", bufs=1, space="PSUM")
            pS = pS_cm.__enter__()
            py_cm = tc.tile_pool(name="py", bufs=2, space="PSUM")
            py = py_cm.__enter__()
            vR = big.tile([128, NCH, 64], bf16)
            khR = big.tile([128, NCH, 64], bf16)
            # prefill k' transposes + plain drains (scan-independent)
            for c in range(NCH):
                csl = slice(c * C, (c + 1) * C)
                tk = po.tile([128, 64], bf16, tag="o")
                nc.tensor.transpose(tk[:], vgp[0:64, csl], identb[0:64, 0:64])
                if c % 2 == 0:
                    nc.vector.tensor_copy(khR[:, c, :], tk[:])
                else:
                    nc.scalar.copy(khR[:, c, :], tk[:])
            hp_cm = tc.high_priority()
            hp_cm.__enter__()
            sim32 = work.tile([32, 128], f32, tag="sim32")
            nk32 = work.tile([32, 128], f32, tag="nk32")
            glog32 = work.tile([32, 128], f32, tag="glog32")
            nv32 = work.tile([32, 128], f32, tag="nv32")
            for i, (dst, r0) in enumerate(((sim32, 0), (nk32, 2), (glog32, 32), (nv32, 34))):
                nc.sync.dma_start(
                    dst[:],
                    sg4[r0:r0 + 2, :].rearrange("p (s t) -> p s t", s=SEG))
            # e and g stacked in eg [32, 2, 128]
            eg = work.tile([32, 2, 128], f32, tag="eg")
            nc.scalar.activation(eg[:, 0, :], sim32[:], AF.Exp, bias=0.0, scale=qs32)
            rtk = work.tile([32, 128], f32, tag="rtk")
            nc.scalar.sqrt(rtk[:], nk32[:])
            rtv = work.tile([32, 128], f32, tag="rtv")
            nc.scalar.sqrt(rtv[:], nv32[:])
            rik = work.tile([32, 128], f32, tag="rik")
            nc.vector.reciprocal(rik[:], rtk[:])
            riv = work.tile([32, 128], f32, tag="riv")
            nc.vector.reciprocal(riv[:], rtv[:])
            nrm = work.tile([32, 128], f32, tag="nrm")
            nc.vector.tensor_mul(nrm[:], rik[:], riv[:])
            glogs = work.tile([32, 128], f32, tag="glogs")
            nc.vector.tensor_mul(glogs[:], glog32[:], nrm[:])
            grelu = work.tile([32, 128], f32, tag="grelu")
            nc.scalar.activation(grelu[:], glogs[:], AF.Relu, bias=gb, scale=1.0)
            gsq = work.tile([32, 128], f32, tag="gsq")
            nc.scalar.square(gsq[:], grelu[:])
            nc.vector.tensor_scalar_add(eg[:, 1, :], gsq[:], EPS)
            ghat = work.tile([32, 128], bf16, tag="ghat")
            nc.vector.tensor_mul(ghat[:], eg[:, 1, :], nrm[:])
            gR_ps = py.tile([128, 32], bf16, tag="y")
            nc.tensor.transpose(gR_ps[:], ghat[:], identb[0:32, 0:32])
            gR = work.tile([128, 32], f32, tag="gR")
            nc.vector.tensor_copy(gR[:], gR_ps[:])
            # cumsums (separate scans, shared dest)
            ecs = work.tile([32, 2, 128], f32, tag="ecs")
            nc.vector.tensor_tensor_scan(ecs[:, 0, :], eg[:, 0, :], z2[:, 0, :],
                                         0.0, OP.add, OP.add)
            nc.vector.tensor_tensor_scan(ecs[:, 1, :], eg[:, 1, :], z2[:, 0, :],
                                         0.0, OP.add, OP.add)
            offs_ps = py.tile([32, 2], f32, tag="y")
            nc.tensor.matmul(offs_ps[:], lt32[:, :], ecs[:, :, 127:128], start=True, stop=True)
            offs = work.tile([32, 2], f32, tag="offs")
            nc.vector.tensor_scalar_add(offs[:], offs_ps[:], EPS)
            sgs = work.tile([32, 2, 128], f32, tag="sgs")
            nc.vector.tensor_scalar(sgs[:, 0, :], ecs[:, 0, :], offs[:, 0:1], None, OP.add)
            nc.scalar.activation(sgs[:, 1, :], ecs[:, 1, :], AF.Identity,
                                 bias=offs[:, 1:2], scale=1.0)
            rsg = work.tile([32, 2, 128], f32, tag="rsg")
            nc.vector.reciprocal(rsg[:], sgs[:])
            sw32 = work.tile([32, 128], f32, tag="sw32")
            nc.vector.tensor_mul(sw32[:], eg[:, 0, :], rsg[:, 0, :])
            coef = work.tile([32, 128], f32, tag="coef")
            if DEBUG:
                sig = work.tile([32, 128], f32, tag="sig")
                nc.scalar.activation(sig[:], sw32[:], AF.Sigmoid, bias=0.0, scale=1.0)
                nc.vector.tensor_mul(coef[:], sw32[:], sig[:])
            else:
                nc.scalar.activation(coef[:], sw32[:], AF.Silu, bias=0.0, scale=1.0)
            w32 = work.tile([32, 128], bf16, tag="w32")
            nc.vector.scalar_tensor_tensor(w32[:], coef[:], 1.0, rsg[:, 1, :],
                                           OP.add, OP.mult)
            # w broadcast -> qw = q * w
            w2 = work.tile([2, L], bf16, tag="w2")
            nc.sync.dma_start(w2.rearrange("p (s t) -> p s t", s=SEG), w32[:])
            qw = big.tile([64, L], bf16)
            qw_cuts = [0, 128, 512, 1024, 1536, 2048]
            for bi in range(len(qw_cuts) - 1):
                bsl = slice(qw_cuts[bi], qw_cuts[bi + 1])
                wbc = py.tile([64, BS], f32, tag="y")
                wn = qw_cuts[bi + 1] - qw_cuts[bi]
                nc.tensor.matmul(wbc[:, 0:wn], ones2[:, :], w2[:, bsl],
                                 start=True, stop=True)
                nc.vector.tensor_mul(qw[:, bsl], qk[0:64, bsl], wbc[:, 0:wn])
            hp_cm.__exit__(None, None, None)
            # prefill v' transposes; drains carry ghat scale
            for c in range(NCH):
                csl = slice(c * C, (c + 1) * C)
                tv = po.tile([128, 64], bf16, tag="o")
                nc.tensor.transpose(tv[:], v0[:, csl], identb[0:64, 0:64])
                if c % 2 == 0:
                    nc.vector.tensor_scalar(vR[:, c, 0:32], tv[:, 0:32],
                                            gR[:, c:c + 1], None, OP.mult)
                else:
                    nc.scalar.activation(vR[:, c, 0:32], tv[:, 0:32], AF.Copy,
                                         bias=0.0, scale=gR[:, c:c + 1])
                nc.vector.tensor_scalar(vR[:, c, 32:64], tv[:, 32:64],
                                        gR[:, 16 + c:16 + c + 1], None, OP.mult)

            # ---- phase C ----
            if True:

                Sf_prev = None
                # S-recurrence first: qw-independent, overlaps scan/w-bcast
                Sall = big.tile([64, NCH, 64], bf16)
                for cidx in range(NCH - 1):
                    dS = pS.tile([64, 64], f32, tag="s")
                    nc.tensor.matmul(dS[:], vR[:, cidx, :], khR[:, cidx, :],
                                     start=True, stop=True)
                    Sf_new = work.tile([64, 64], f32, tag="Sf")
                    if cidx == 0:
                        nc.vector.tensor_copy(Sf_new[:], dS[:])
                    else:
                        nc.vector.tensor_add(Sf_new[:], Sf_prev[:], dS[:])
                    Sf_prev = Sf_new
                    nc.gpsimd.tensor_copy(Sall[:, cidx, :], Sf_new[:])
                # main loop: fully pipelineable
                for cidx in range(NCH):
                    csl = slice(cidx * C, (cidx + 1) * C)
                    atm = work.tile([128, 2, 128], bf16, tag="atm")
                    for lane in range(2):
                        lsl = slice(lane * 32, lane * 32 + 32)
                        ppl = po.tile([128, 128], f32, tag="o")
                        nc.tensor.matmul(ppl[:], v0[lsl, csl], qw[lsl, csl],
                                         start=True, stop=True)
                        if lane == 0:
                            nc.vector.scalar_tensor_tensor(
                                atm[:, 0, :], ppl[:], gR[:, cidx:cidx + 1],
                                mask2[:, 0, :], OP.mult, OP.mult)
                        else:
                            p1sb = work.tile([128, 128], bf16, tag="p1sb")
                            nc.scalar.activation(
                                p1sb[:], ppl[:], AF.Copy, bias=0.0,
                                scale=gR[:, 16 + cidx:16 + cidx + 1])
                            nc.gpsimd.tensor_mul(atm[:, 1, :], p1sb[:],
                                                 mask2[:, 1, :])
                    # ctT = khR^T atm + S^T qw   [64 (lane,e), 128 j]
                    ctT_ps = po.tile([64, 128], f32, tag="o")
                    for lane in range(2):
                        lsl = slice(lane * 32, lane * 32 + 32)
                        nc.tensor.matmul(ctT_ps[lsl, :], khR[:, cidx, lsl],
                                         atm[:, lane, :],
                                         start=True, stop=(cidx == 0))
                        if cidx > 0:
                            nc.tensor.matmul(
                                ctT_ps[lsl, :],
                                Sall[lsl, cidx - 1, lane * 32:lane * 32 + 32],
                                qw[lsl, csl],
                                start=False, stop=True)
                    ctT = outp.tile([64, 128], bf16, tag="ctT")
                    if cidx % 2 == 0:
                        nc.scalar.copy(ctT[:], ctT_ps[:])
                    else:
                        nc.vector.tensor_copy(ctT[:], ctT_ps[:])
                    # y
                    if cidx % 2 == 0:
                        y_ps = py.tile([128, 2, 256], f32, tag="y")
                    nc.tensor.matmul(y_ps[:, cidx % 2, :], ctT[:], woT[0:64, :],
                                     start=True, stop=True)
                    if cidx % 2 == 1:
                        y_sb = outp.tile([128, 2, 256], bf16, tag="ysb")
                        nc.scalar.copy(y_sb[:, 0, :], y_ps[:, 0, :])
                        nc.vector.tensor_copy(y_sb[:, 1, :], y_ps[:, 1, :])
                        nc.sync.dma_start(
                            y_d[(cidx - 1) * C:(cidx + 1) * C, :].rearrange(
                                "(a p) d -> p a d", a=2),
                            y_sb[:])
                if DEBUG:
                    nc.sync.dma_start(dqk_d[:, :], qk[:])
                    nc.sync.dma_start(dvgp_d[:, :], vgp[:])
                    dsc = const.tile([32, 128, 6], f32, tag="dsc")
                    nc.vector.tensor_copy(dsc[:, :, 0], sim32[:])
                    nc.vector.tensor_copy(dsc[:, :, 1], nk32[:])
                    nc.vector.tensor_copy(dsc[:, :, 2], nv32[:])
                    nc.vector.tensor_copy(dsc[:, :, 3], glog32[:])
                    nc.vector.tensor_copy(dsc[:, :, 4], eg[:, 1, :])
                    nc.vector.tensor_copy(dsc[:, :, 5], w32[:])
                    nc.sync.dma_start(dsc_d[:, :, :], dsc[:])
                    nc.sync.dma_start(dkh_d[:, :], khR[:].rearrange("p a b -> p (a b)"))
                    nc.sync.dma_start(dqw_d[:, :], qw[:])
            py_cm.__exit__(None, None, None)
            pS_cm.__exit__(None, None, None)
            po_cm.__exit__(None, None, None)

    global _LAST_TC_SPAN
    try:
        _LAST_TC_SPAN = max(e[2] for e in tc._perfetto_entries if e[2] is not None)
    except Exception:
        _LAST_TC_SPAN = 0
    nc.compile()
    return nc


_NC_CACHE = None
_LAST_IN_MAPS = None
_LAST_TC_SPAN = 0


def _get_nc():
    global _NC_CACHE
    if _NC_CACHE is None:
        _NC_CACHE = _build_bass()
    return _NC_CACHE


def _bf16(a):
    import ml_dtypes
    return np.asarray(a, dtype=np.float32).astype(ml_dtypes.bfloat16)


def kernel(**inputs):
    import sys
    if '/opt/trn_rl_repo' not in sys.path:
        sys.path.insert(0, '/opt/trn_rl_repo')
    from concourse.bass_utils import run_bass_kernel_spmd

    inp = {k: np.asarray(v) for k, v in inputs.items()}
    x = inp['x'].astype(np.float32)
    wq, wk, wv, wo = (inp[n].astype(np.float32) for n in ('wq_w', 'wk_w', 'wv_w', 'wo_w'))
    wg = inp['wg_w'].astype(np.float32).reshape(HD, HD)
    gbv = float(inp['wg_b'][0])
    kvs = inp['kv_norm_scale'].astype(np.float32)[0, :, 0]
    qks = inp['qk_norm_scale'].astype(np.float32)[0, :, 0]

    nc = _get_nc()

    identb = np.eye(128, dtype=np.float32)
    maskc = (np.arange(128)[:, None] <= np.arange(128)[None, :]).astype(np.float32)
    lt32 = np.zeros((32, 32), np.float32)
    for p in range(32):
        for s in range(32):
            if p // 16 == s // 16 and s % 16 < p % 16:
                lt32[s, p] = 1.0

    in_maps = []
    for core in range(N_CORES):
        b = core // 4
        heads = (2 * (core % 4), 2 * (core % 4) + 1)
        xT = np.ascontiguousarray(
            x[b].T.reshape(2, 128, L).transpose(1, 0, 2))  # [128,2,L]

        a_v = np.empty((2, HD), np.float32)
        b_v = np.empty((2, HD), np.float32)
        mg = []
        for li, hh in enumerate(heads):
            sc = kvs[hh]
            a_v[li] = sc[:, 0]
            b_v[li] = sc[0, :] / sc[0, 0]
            mg.append(wg * sc)

        # W1 = {q, v'}; W2 = {k', gp}
        W1 = np.empty((128, D), np.float32)
        W2 = np.empty((128, D), np.float32)
        for li, hh in enumerate(heads):
            W1[li * 32:li * 32 + 32] = wq[hh * HD:(hh + 1) * HD, :]
            W1[64 + li * 32:64 + li * 32 + 32] = a_v[li][:, None] * wv[hh * HD:(hh + 1) * HD, :]
            W2[li * 32:li * 32 + 32] = b_v[li][:, None] * wk[hh * HD:(hh + 1) * HD, :]
            W2[64 + li * 32:64 + li * 32 + 32] = (
                (1.0 / a_v[li])[:, None] * (mg[li] @ wk[hh * HD:(hh + 1) * HD, :]))
        w1T = np.ascontiguousarray(W1.T.reshape(2, 128, 128).transpose(1, 0, 2))
        w2T = np.ascontiguousarray(W2.T.reshape(2, 128, 128).transpose(1, 0, 2))

        cw1 = np.zeros((128, 32), np.float32)
        cw2 = np.zeros((128, 32), np.float32)
        for li in range(2):
            cw1[li * 32:(li + 1) * 32, li] = 1.0 / b_v[li]                   # sim
            cw1[64 + li * 32:64 + (li + 1) * 32, 2 + li] = 1.0 / b_v[li] ** 2  # |k|^2
            cw2[64 + li * 32:64 + (li + 1) * 32, li] = 1.0                   # glog
            cw2[li * 32:(li + 1) * 32, 2 + li] = 1.0 / a_v[li] ** 2          # |v|^2

        woT = np.empty((64, D), np.float32)
        for li, hh in enumerate(heads):
            woT[li * 32:(li + 1) * 32, :] = wo[:, hh * HD:(hh + 1) * HD].T

        ones2 = np.zeros((2, 64), np.float32)
        ones2[0, 0:32] = 1.0
        ones2[1, 32:64] = 1.0

        wb = np.zeros((128, 1280), np.float32)
        wb[:, 0:256] = w1T.reshape(128, 256)
        wb[:, 256:512] = w2T.reshape(128, 256)
        wb[:, 512:544] = cw1
        wb[:, 544:576] = cw2
        wb[0:64, 576:832] = woT
        wb[:, 832:960] = identb
        wb[:, 960:1088] = maskc
        wb[:, 1088:1216] = maskc
        wb[0:2, 1216:1280] = ones2

        sbm = np.zeros((32, 34), np.float32)
        sbm[0:16, 0] = qks[heads[0]]
        sbm[16:32, 0] = qks[heads[1]]
        sbm[:, 1] = gbv
        sbm[:, 2:34] = lt32

        in_maps.append({"xT": _bf16(xT), "wb": _bf16(wb), "sb": sbm})

    global _LAST_IN_MAPS
    _LAST_IN_MAPS = in_maps
    res = run_bass_kernel_spmd(nc, in_maps, core_ids=list(range(N_CORES)))
    out = np.zeros((B, L, D), np.float32)
    for core in range(N_CORES):
        out[core // 4] += np.asarray(res.results[core]["y"], dtype=np.float32)
    out += inp['wo_b'].astype(np.float32)[None, None, :]
    return out


# revision 9
# speedup vs baseline: 1.0382x; 1.0382x over previous
import numpy as np

EPS = 1e-5

# nn_AssociativeAttention: B=2, L=2048, D=256, H=8, h=32.
# 8 cores x 2 (b,h) lanes each. bf16 matmul pipeline, v3:
#  A: fused {q,v'} {k',gp} projections [d,l]; same-base products; colsum
#     matmuls -> sim/|k|^2/|v|^2/glog per position (stacked [64,BS] psum)
#  B: cumsum-only scan chain in [32,128] (max-scan dropped; s_scan>=1),
#     w broadcast via PE ones-matmul -> qw = q*w
#  C: chunked causal linear attention in transposed-ct form:
#     ctT[e,j] = sum_i khR[i,e] atm[i,j] + sum_d S[d,e] qw[d,j]
#     with atm = mask * P', P' = v'^T qw; ghat folded into khR drains;
#     dS accumulated in PSUM. Partial out-proj y (bf16) per core.
# Host reduces 4 cores per batch and adds wo_b.

B, L, D, H, HD = 2, 2048, 256, 8, 32
N_CORES = 8
C = 128
NCH = L // C
SEG = 16
NBLK = 4
BS = L // NBLK  # 512


def _build_bass():
    import os as _os
    DEBUG = int(_os.environ.get('KV2_DEBUG', '0'))
    SAFES = int(_os.environ.get('KV2_SAFES', '0'))
    import sys
    if '/opt/trn_rl_repo' not in sys.path:
        sys.path.insert(0, '/opt/trn_rl_repo')
    from contextlib import ExitStack
    import concourse.bacc as bacc
    import concourse.tile as tile
    from concourse import mybir

    f32 = mybir.dt.float32
    bf16 = mybir.dt.bfloat16
    AF = mybir.ActivationFunctionType
    OP = mybir.AluOpType

    nc = bacc.Bacc()
    dt = nc.dram_tensor
    xT_d = dt("xT", [128, 2, L], bf16, kind="ExternalInput")
    # blob cols: w1T(0:256) w2T(256:512) cw1(512:544) cw2(544:576)
    #   woT(576:832, rows 0:64) identb(832:960) mask2(960:1216, 2x128)
    #   ones2(1216:1280, rows 0:2)
    wb_d = dt("wb", [128, 1280], bf16, kind="ExternalInput")
    sb_d = dt("sb", [32, 34], f32, kind="ExternalInput")  # qs|gb|lt32
    y_d = dt("y", [L, D], bf16, kind="ExternalOutput")
    if DEBUG:
        dqk_d = dt("dqk", [128, L], bf16, kind="ExternalOutput")
        dvgp_d = dt("dvgp", [128, L], bf16, kind="ExternalOutput")
        dsc_d = dt("dsc", [32, 128, 6], f32, kind="ExternalOutput")
        dkh_d = dt("dkh", [128, NCH * 64], bf16, kind="ExternalOutput")
        dqw_d = dt("dqw", [64, L], bf16, kind="ExternalOutput")

    with tile.TileContext(nc) as tc:
        with ExitStack() as ctx:
            const = ctx.enter_context(tc.tile_pool(name="const", bufs=1))
            big = ctx.enter_context(tc.tile_pool(name="big", bufs=1))
            work = ctx.enter_context(tc.tile_pool(name="work", bufs=2))
            outp = ctx.enter_context(tc.tile_pool(name="outp", bufs=3))

            wb = const.tile([128, 1280], bf16)
            nc.sync.dma_start(wb[:], wb_d[:, :])
            sb = const.tile([32, 34], f32)
            nc.sync.dma_start(sb[:], sb_d[:, :])
            w1T = wb[:, 0:256].rearrange("p (f m) -> p f m", f=2)
            w2T = wb[:, 256:512].rearrange("p (f m) -> p f m", f=2)
            cw1 = wb[:, 512:544]
            cw2 = wb[:, 544:576]
            woT = wb[:, 576:832]
            identb = wb[:, 832:960]
            mask2 = wb[:, 960:1216].rearrange("p (a t) -> p a t", a=2)
            ones2 = wb[0:2, 1216:1280]
            qs32 = sb[:, 0:1]
            gb = sb[:, 1:2]
            lt32 = sb[:, 2:34]

            xT = big.tile([128, 2, L], bf16)
            for blk in range(NBLK):
                bsl = slice(blk * BS, (blk + 1) * BS)
                nc.sync.dma_start(xT[:, :, bsl], xT_d[:, :, bsl])

            z2 = const.tile([32, 2, 128], f32)
            nc.vector.memset(z2[:], 0.0)

            # ---- phase A ----
            qk = big.tile([128, L], bf16)    # q 0:64, v' 64:128
            vgp = big.tile([128, L], bf16)   # k' 0:64, gp 64:128
            v0 = big.tile([64, L], bf16)     # v' shifted to base 0
            sg4 = big.tile([64, L], f32)     # rows 0:4 sim/nk, 32:36 glog/nv
            with tc.tile_pool(name="pmm", bufs=2, space="PSUM") as pmm, \
                 tc.tile_pool(name="psmall", bufs=2, space="PSUM") as psmall:
                for blk in range(NBLK):
                    bsl = slice(blk * BS, (blk + 1) * BS)
                    ps1 = pmm.tile([128, BS], f32, tag="mm")
                    for f in range(2):
                        nc.tensor.matmul(ps1[:], w1T[:, f, :], xT[:, f, bsl],
                                         start=(f == 0), stop=(f == 1))
                    nc.scalar.copy(qk[:, bsl], ps1[:])
                    ps2 = pmm.tile([128, BS], f32, tag="mm")
                    for f in range(2):
                        nc.tensor.matmul(ps2[:], w2T[:, f, :], xT[:, f, bsl],
                                         start=(f == 0), stop=(f == 1))
                    nc.vector.tensor_copy(vgp[:, bsl], ps2[:])
                    s1 = work.tile([128, BS], bf16, tag="s1")
                    nc.vector.tensor_mul(s1[0:64, :], qk[0:64, bsl], vgp[0:64, bsl])
                    nc.gpsimd.tensor_mul(s1[64:128, :], vgp[0:64, bsl], vgp[0:64, bsl])
                    s2 = work.tile([128, BS], bf16, tag="s2")
                    nc.gpsimd.tensor_mul(s2[0:64, :], qk[64:128, bsl], qk[64:128, bsl])
                    nc.vector.tensor_mul(s2[64:128, :], qk[64:128, bsl], vgp[64:128, bsl])
                    cs = psmall.tile([64, BS], f32, tag="cs")
                    hpA = tc.high_priority()
                    hpA.__enter__()
                    nc.tensor.matmul(cs[0:32, :], cw1[:, :], s1[:], start=True, stop=True)
                    nc.tensor.matmul(cs[32:64, :], cw2[:, :], s2[:], start=True, stop=True)
                    if blk % 2 == 0:
                        nc.scalar.copy(sg4[:, bsl], cs[:])
                    else:
                        nc.vector.tensor_copy(sg4[:, bsl], cs[:])
                    hpA.__exit__(None, None, None)
                    nc.sync.dma_start(v0[:, bsl], qk[64:128, bsl])

            # ---- phase B ----
            po_cm = tc.tile_pool(name="po", bufs=5, space="PSUM")
            po = po_cm.__enter__()
            pS_cm = tc.tile_pool(name="pS", bufs=1, space="PSUM")
            pS = pS_cm.__enter__()
            py_cm = tc.tile_pool(name="py", bufs=2, space="PSUM")
            py = py_cm.__enter__()
            vR = big.tile([128, NCH, 64], bf16)
            khR = big.tile([128, NCH, 64], bf16)
            # prefill k' transposes + plain drains (scan-independent)
            for c in range(NCH):
                csl = slice(c * C, (c + 1) * C)
                tk = po.tile([128, 64], bf16, tag="o")
                nc.tensor.transpose(tk[:], vgp[0:64, csl], identb[0:64, 0:64])
                if c % 2 == 0:
                    nc.vector.tensor_copy(khR[:, c, :], tk[:])
                else:
                    nc.scalar.copy(khR[:, c, :], tk[:])
            hp_cm = tc.high_priority()
            hp_cm.__enter__()
            sim32 = work.tile([32, 128], f32, tag="sim32")
            nk32 = work.tile([32, 128], f32, tag="nk32")
            glog32 = work.tile([32, 128], f32, tag="glog32")
            nv32 = work.tile([32, 128], f32, tag="nv32")
            for i, (dst, r0) in enumerate(((sim32, 0), (nk32, 2), (glog32, 32), (nv32, 34))):
                nc.sync.dma_start(
                    dst[:],
                    sg4[r0:r0 + 2, :].rearrange("p (s t) -> p s t", s=SEG))
            # e and g stacked in eg [32, 2, 128]
            eg = work.tile([32, 2, 128], f32, tag="eg")
            nc.scalar.activation(eg[:, 0, :], sim32[:], AF.Exp, bias=0.0, scale=qs32)
            rtk = work.tile([32, 128], f32, tag="rtk")
            nc.scalar.sqrt(rtk[:], nk32[:])
            rtv = work.tile([32, 128], f32, tag="rtv")
            nc.scalar.sqrt(rtv[:], nv32[:])
            rik = work.tile([32, 128], f32, tag="rik")
            nc.vector.reciprocal(rik[:], rtk[:])
            riv = work.tile([32, 128], f32, tag="riv")
            nc.vector.reciprocal(riv[:], rtv[:])
            nrm = work.tile([32, 128], f32, tag="nrm")
            nc.vector.tensor_mul(nrm[:], rik[:], riv[:])
            glogs = work.tile([32, 128], f32, tag="glogs")
            nc.vector.tensor_mul(glogs[:], glog32[:], nrm[:])
            grelu = work.tile([32, 128], f32, tag="grelu")
            nc.scalar.activation(grelu[:], glogs[:], AF.Relu, bias=gb, scale=1.0)
            gsq = work.tile([32, 128], f32, tag="gsq")
            nc.scalar.square(gsq[:], grelu[:])
            nc.vector.tensor_scalar_add(eg[:, 1, :], gsq[:], EPS)
            ghat = work.tile([32, 128], bf16, tag="ghat")
            nc.vector.tensor_mul(ghat[:], eg[:, 1, :], nrm[:])
            gR_ps = py.tile([128, 32], bf16, tag="y")
            nc.tensor.transpose(gR_ps[:], ghat[:], identb[0:32, 0:32])
            gR = work.tile([128, 32], f32, tag="gR")
            nc.vector.tensor_copy(gR[:], gR_ps[:])
            # cumsums (separate scans, shared dest)
            ecs = work.tile([32, 2, 128], f32, tag="ecs")
            nc.vector.tensor_tensor_scan(ecs[:, 0, :], eg[:, 0, :], z2[:, 0, :],
                                         0.0, OP.add, OP.add)
            nc.vector.tensor_tensor_scan(ecs[:, 1, :], eg[:, 1, :], z2[:, 0, :],
                                         0.0, OP.add, OP.add)
            offs_ps = py.tile([32, 2], f32, tag="y")
            nc.tensor.matmul(offs_ps[:], lt32[:, :], ecs[:, :, 127:128], start=True, stop=True)
            offs = work.tile([32, 2], f32, tag="offs")
            nc.vector.tensor_scalar_add(offs[:], offs_ps[:], EPS)
            sgs = work.tile([32, 2, 128], f32, tag="sgs")
            nc.vector.tensor_scalar(sgs[:, 0, :], ecs[:, 0, :], offs[:, 0:1], None, OP.add)
            nc.scalar.activation(sgs[:, 1, :], ecs[:, 1, :], AF.Identity,
                                 bias=offs[:, 1:2], scale=1.0)
            rsg = work.tile([32, 2, 128], f32, tag="rsg")
            nc.vector.reciprocal(rsg[:], sgs[:])
            sw32 = work.tile([32, 128], f32, tag="sw32")
            nc.vector.tensor_mul(sw32[:], eg[:, 0, :], rsg[:, 0, :])
            coef = work.tile([32, 128], f32, tag="coef")
            if DEBUG:
                sig = work.tile([32, 128], f32, tag="sig")
                nc.scalar.activation(sig[:], sw32[:], AF.Sigmoid, bias=0.0, scale=1.0)
                nc.vector.tensor_mul(coef[:], sw32[:], sig[:])
            else:
                nc.scalar.activation(coef[:], sw32[:], AF.Silu, bias=0.0, scale=1.0)
            w32 = work.tile([32, 128], bf16, tag="w32")
            nc.vector.scalar_tensor_tensor(w32[:], coef[:], 1.0, rsg[:, 1, :],
                                           OP.add, OP.mult)
            # w broadcast -> qw = q * w
            w2 = work.tile([2, L], bf16, tag="w2")
            nc.sync.dma_start(w2.rearrange("p (s t) -> p s t", s=SEG), w32[:])
            qw = big.tile([64, L], bf16)
            qw_cuts = [0, 128, 512, 1024, 1536, 2048]
            for bi in range(len(qw_cuts) - 1):
                bsl = slice(qw_cuts[bi], qw_cuts[bi + 1])
                wbc = py.tile([64, BS], f32, tag="y")
                wn = qw_cuts[bi + 1] - qw_cuts[bi]
                nc.tensor.matmul(wbc[:, 0:wn], ones2[:, :], w2[:, bsl],
                                 start=True, stop=True)
                nc.vector.tensor_mul(qw[:, bsl], qk[0:64, bsl], wbc[:, 0:wn])
            hp_cm.__exit__(None, None, None)
            # prefill v' transposes; drains carry ghat scale
            for c in range(NCH):
                csl = slice(c * C, (c + 1) * C)
                tv = po.tile([128, 64], bf16, tag="o")
                nc.tensor.transpose(tv[:], v0[:, csl], identb[0:64, 0:64])
                if c % 2 == 0:
                    nc.vector.tensor_scalar(vR[:, c, 0:32], tv[:, 0:32],
                                            gR[:, c:c + 1], None, OP.mult)
                else:
                    nc.scalar.activation(vR[:, c, 0:32], tv[:, 0:32], AF.Copy,
                                         bias=0.0, scale=gR[:, c:c + 1])
                nc.vector.tensor_scalar(vR[:, c, 32:64], tv[:, 32:64],
                                        gR[:, 16 + c:16 + c + 1], None, OP.mult)

            # ---- phase C ----
            if True:

                Sf_prev = None
                # S-recurrence first: qw-independent, overlaps scan/w-bcast
                Sall = big.tile([64, NCH, 64], bf16)
                for cidx in range(NCH - 1):
                    dS = pS.tile([64, 64], f32, tag="s")
                    nc.tensor.matmul(dS[:], vR[:, cidx, :], khR[:, cidx, :],
                                     start=True, stop=True)
                    Sf_new = work.tile([64, 64], f32, tag="Sf")
                    if cidx == 0:
                        nc.vector.tensor_copy(Sf_new[:], dS[:])
                    else:
                        nc.vector.tensor_add(Sf_new[:], Sf_prev[:], dS[:])
                    Sf_prev = Sf_new
                    nc.gpsimd.tensor_copy(Sall[:, cidx, :], Sf_new[:])
                # main loop: fully pipelineable
                for cidx in range(NCH):
                    csl = slice(cidx * C, (cidx + 1) * C)
                    atm = work.tile([128, 2, 128], bf16, tag="atm")
                    for lane in range(2):
                        lsl = slice(lane * 32, lane * 32 + 32)
                        ppl = po.tile([128, 128], f32, tag="o")
                        nc.tensor.matmul(ppl[:], v0[lsl, csl], qw[lsl, csl],
                                         start=True, stop=True)
                        if lane == 0:
                            nc.vector.scalar_tensor_tensor(
                                atm[:, 0, :], ppl[:], gR[:, cidx:cidx + 1],
                                mask2[:, 0, :], OP.mult, OP.mult)
                        else:
                            p1sb = work.tile([128, 128], bf16, tag="p1sb")
                            nc.scalar.activation(
                                p1sb[:], ppl[:], AF.Copy, bias=0.0,
                                scale=gR[:, 16 + cidx:16 + cidx + 1])
                            nc.gpsimd.tensor_mul(atm[:, 1, :], p1sb[:],
                                                 mask2[:, 1, :])
                    # ctT = khR^T atm + S^T qw   [64 (lane,e), 128 j]
                    ctT_ps = po.tile([64, 128], f32, tag="o")
                    for lane in range(2):
                        lsl = slice(lane * 32, lane * 32 + 32)
                        nc.tensor.matmul(ctT_ps[lsl, :], khR[:, cidx, lsl],
                                         atm[:, lane, :],
                                         start=True, stop=(cidx == 0))
                        if cidx > 0:
                            nc.tensor.matmul(
                                ctT_ps[lsl, :],
                                Sall[lsl, cidx - 1, lane * 32:lane * 32 + 32],
                                qw[lsl, csl],
                                start=False, stop=True)
                    ctT = outp.tile([64, 128], bf16, tag="ctT")
                    if cidx % 2 == 0:
                        nc.scalar.copy(ctT[:], ctT_ps[:])
                    else:
                        nc.vector.tensor_copy(ctT[:], ctT_ps[:])
                    # y
                    if cidx % 2 == 0:
                        y_ps = py.tile([128, 2, 256], f32, tag="y")
                    nc.tensor.matmul(y_ps[:, cidx % 2, :], ctT[:], woT[0:64, :],
                                     start=True, stop=True)
                    if cidx % 2 == 1:
                        y_sb = outp.tile([128, 2, 256], bf16, tag="ysb")
                        nc.scalar.copy(y_sb[:, 0, :], y_ps[:, 0, :])
                        nc.vector.tensor_copy(y_sb[:, 1, :], y_ps[:, 1, :])
                        nc.sync.dma_start(
                            y_d[(cidx - 1) * C:(cidx + 1) * C, :].rearrange(
                                "(a p) d -> p a d", a=2),
                            y_sb[:])
                if DEBUG:
                    nc.sync.dma_start(dqk_d[:, :], qk[:])
                    nc.sync.dma_start(dvgp_d[:, :], vgp[:])
                    dsc = const.tile([32, 128, 6], f32, tag="dsc")
                    nc.vector.tensor_copy(dsc[:, :, 0], sim32[:])
                    nc.vector.tensor_copy(dsc[:, :, 1], nk32[:])
                    nc.vector.tensor_copy(dsc[:, :, 2], nv32[:])
                    nc.vector.tensor_copy(dsc[:, :, 3], glog32[:])
                    nc.vector.tensor_copy(dsc[:, :, 4], eg[:, 1, :])
                    nc.vector.tensor_copy(dsc[:, :, 5], w32[:])
                    nc.sync.dma_start(dsc_d[:, :, :], dsc[:])
                    nc.sync.dma_start(dkh_d[:, :], khR[:].rearrange("p a b -> p (a b)"))
                    nc.sync.dma_start(dqw_d[:, :], qw[:])
            py_cm.__exit__(None, None, None)
            pS_cm.__exit__(None, None, None)
            po_cm.__exit__(None, None, None)

    global _LAST_TC_SPAN
    try:
        _LAST_TC_SPAN = max(e[2] for e in tc._perfetto_entries if e[2] is not None)
    except Exception:
        _LAST_TC_SPAN = 0
    nc.compile()
    return nc


_NC_CACHE = None
_LAST_IN_MAPS = None
_LAST_TC_SPAN = 0


def _get_nc():
    global _NC_CACHE
    if _NC_CACHE is None:
        _NC_CACHE = _build_bass()
    return _NC_CACHE


def _bf16(a):
    import ml_dtypes
    return np.asarray(a, dtype=np.float32).astype(ml_dtypes.bfloat16)


def kernel(**inputs):
    import sys
    if '/opt/trn_rl_repo' not in sys.path:
        sys.path.insert(0, '/opt/trn_rl_repo')
    from concourse.bass_utils import run_bass_kernel_spmd

    inp = {k: np.asarray(v) for k, v in inputs.items()}
    x = inp['x'].astype(np.float32)
    wq, wk, wv, wo = (inp[n].astype(np.float32) for n in ('wq_w', 'wk_w', 'wv_w', 'wo_w'))
    wg = inp['wg_w'].astype(np.float32).reshape(HD, HD)
    gbv = float(inp['wg_b'][0])
    kvs = inp['kv_norm_scale'].astype(np.float32)[0, :, 0]
    qks = inp['qk_norm_scale'].astype(np.float32)[0, :, 0]

    nc = _get_nc()

    identb = np.eye(128, dtype=np.float32)
    maskc = (np.arange(128)[:, None] <= np.arange(128)[None, :]).astype(np.float32)
    lt32 = np.zeros((32, 32), np.float32)
    for p in range(32):
        for s in range(32):
            if p // 16 == s // 16 and s % 16 < p % 16:
                lt32[s, p] = 1.0

    in_maps = []
    for core in range(N_CORES):
        b = core // 4
        heads = (2 * (core % 4), 2 * (core % 4) + 1)
        xT = np.ascontiguousarray(
            x[b].T.reshape(2, 128, L).transpose(1, 0, 2))  # [128,2,L]

        a_v = np.empty((2, HD), np.float32)
        b_v = np.empty((2, HD), np.float32)
        mg = []
        for li, hh in enumerate(heads):
            sc = kvs[hh]
            a_v[li] = sc[:, 0]
            b_v[li] = sc[0, :] / sc[0, 0]
            mg.append(wg * sc)

        # W1 = {q, v'}; W2 = {k', gp}
        W1 = np.empty((128, D), np.float32)
        W2 = np.empty((128, D), np.float32)
        for li, hh in enumerate(heads):
            W1[li * 32:li * 32 + 32] = wq[hh * HD:(hh + 1) * HD, :]
            W1[64 + li * 32:64 + li * 32 + 32] = a_v[li][:, None] * wv[hh * HD:(hh + 1) * HD, :]
            W2[li * 32:li * 32 + 32] = b_v[li][:, None] * wk[hh * HD:(hh + 1) * HD, :]
            W2[64 + li * 32:64 + li * 32 + 32] = (
                (1.0 / a_v[li])[:, None] * (mg[li] @ wk[hh * HD:(hh + 1) * HD, :]))
        w1T = np.ascontiguousarray(W1.T.reshape(2, 128, 128).transpose(1, 0, 2))
        w2T = np.ascontiguousarray(W2.T.reshape(2, 128, 128).transpose(1, 0, 2))

        cw1 = np.zeros((128, 32), np.float32)
        cw2 = np.zeros((128, 32), np.float32)
        for li in range(2):
            cw1[li * 32:(li + 1) * 32, li] = 1.0 / b_v[li]                   # sim
            cw1[64 + li * 32:64 + (li + 1) * 32, 2 + li] = 1.0 / b_v[li] ** 2  # |k|^2
            cw2[64 + li * 32:64 + (li + 1) * 32, li] = 1.0                   # glog
            cw2[li * 32:(li + 1) * 32, 2 + li] = 1.0 / a_v[li] ** 2          # |v|^2

        woT = np.empty((64, D), np.float32)
        for li, hh in enumerate(heads):
            woT[li * 32:(li + 1) * 32, :] = wo[:, hh * HD:(hh + 1) * HD].T

        ones2 = np.zeros((2, 64), np.float32)
        ones2[0, 0:32] = 1.0
        ones2[1, 32:64] = 1.0

        wb = np.zeros((128, 1280), np.float32)
        wb[:, 0:256] = w1T.reshape(128, 256)
        wb[:, 256:512] = w2T.reshape(128, 256)
        wb[:, 512:544] = cw1
        wb[:, 544:576] = cw2
        wb[0:64, 576:832] = woT
        wb[:, 832:960] = identb
        wb[:, 960:1088] = maskc
        wb[:, 1088:1216] = maskc
        wb[0:2, 1216:1280] = ones2

        sbm = np.zeros((32, 34), np.float32)
        sbm[0:16, 0] = qks[heads[0]]
        sbm[16:32, 0] = qks[heads[1]]
        sbm[:, 1] = gbv
        sbm[:, 2:34] = lt32

        in_maps.append({"xT": _bf16(xT), "wb": _bf16(wb), "sb": sbm})

    global _LAST_IN_MAPS
    _LAST_IN_MAPS = in_maps
    res = run_bass_kernel_spmd(nc, in_maps, core_ids=list(range(N_CORES)))
    out = np.zeros((B, L, D), np.float32)
    for core in range(N_CORES):
        out[core // 4] += np.asarray(res.results[core]["y"], dtype=np.float32)
    out += inp['wo_b'].astype(np.float32)[None, None, :]
    return out


# revision 10
# speedup vs baseline: 1.0398x; 1.0015x over previous
import numpy as np

EPS = 1e-5

# nn_AssociativeAttention: B=2, L=2048, D=256, H=8, h=32.
# 8 cores x 2 (b,h) lanes each. bf16 matmul pipeline, v3:
#  A: fused {q,v'} {k',gp} projections [d,l]; same-base products; colsum
#     matmuls -> sim/|k|^2/|v|^2/glog per position (stacked [64,BS] psum)
#  B: cumsum-only scan chain in [32,128] (max-scan dropped; s_scan>=1),
#     w broadcast via PE ones-matmul -> qw = q*w
#  C: chunked causal linear attention in transposed-ct form:
#     ctT[e,j] = sum_i khR[i,e] atm[i,j] + sum_d S[d,e] qw[d,j]
#     with atm = mask * P', P' = v'^T qw; ghat folded into khR drains;
#     dS accumulated in PSUM. Partial out-proj y (bf16) per core.
# Host reduces 4 cores per batch and adds wo_b.

B, L, D, H, HD = 2, 2048, 256, 8, 32
N_CORES = 8
C = 128
NCH = L // C
SEG = 16
NBLK = 4
BS = L // NBLK  # 512


def _build_bass():
    import os as _os
    DEBUG = int(_os.environ.get('KV2_DEBUG', '0'))
    SAFES = int(_os.environ.get('KV2_SAFES', '0'))
    import sys
    if '/opt/trn_rl_repo' not in sys.path:
        sys.path.insert(0, '/opt/trn_rl_repo')
    from contextlib import ExitStack
    import concourse.bacc as bacc
    import concourse.tile as tile
    from concourse import mybir

    f32 = mybir.dt.float32
    bf16 = mybir.dt.bfloat16
    AF = mybir.ActivationFunctionType
    OP = mybir.AluOpType

    nc = bacc.Bacc()
    dt = nc.dram_tensor
    xT_d = dt("xT", [128, 2, L], bf16, kind="ExternalInput")
    # blob cols: w1T(0:256) w2T(256:512) cw1(512:544) cw2(544:576)
    #   woT(576:832, rows 0:64) identb(832:960) mask2(960:1216, 2x128)
    #   ones2(1216:1280, rows 0:2)
    wb_d = dt("wb", [128, 1280], bf16, kind="ExternalInput")
    sb_d = dt("sb", [32, 34], f32, kind="ExternalInput")  # qs|gb|lt32
    y_d = dt("y", [L, D], bf16, kind="ExternalOutput")
    if DEBUG:
        dqk_d = dt("dqk", [128, L], bf16, kind="ExternalOutput")
        dvgp_d = dt("dvgp", [128, L], bf16, kind="ExternalOutput")
        dsc_d = dt("dsc", [32, 128, 6], f32, kind="ExternalOutput")
        dkh_d = dt("dkh", [128, NCH * 64], bf16, kind="ExternalOutput")
        dqw_d = dt("dqw", [64, L], bf16, kind="ExternalOutput")

    with tile.TileContext(nc) as tc:
        with ExitStack() as ctx:
            const = ctx.enter_context(tc.tile_pool(name="const", bufs=1))
            big = ctx.enter_context(tc.tile_pool(name="big", bufs=1))
            work = ctx.enter_context(tc.tile_pool(name="work", bufs=2))
            outp = ctx.enter_context(tc.tile_pool(name="outp", bufs=3))

            wb = const.tile([128, 1280], bf16)
            nc.scalar.dma_start(wb[:], wb_d[:, :])
            sb = const.tile([32, 34], f32)
            nc.sync.dma_start(sb[:], sb_d[:, :])
            w1T = wb[:, 0:256].rearrange("p (f m) -> p f m", f=2)
            w2T = wb[:, 256:512].rearrange("p (f m) -> p f m", f=2)
            cw1 = wb[:, 512:544]
            cw2 = wb[:, 544:576]
            woT = wb[:, 576:832]
            identb = wb[:, 832:960]
            mask2 = wb[:, 960:1216].rearrange("p (a t) -> p a t", a=2)
            ones2 = wb[0:2, 1216:1280]
            qs32 = sb[:, 0:1]
            gb = sb[:, 1:2]
            lt32 = sb[:, 2:34]

            xT = big.tile([128, 2, L], bf16)
            for blk, eng in zip(range(NBLK), (nc.sync, nc.sync, nc.sync, nc.scalar)):
                bsl = slice(blk * BS, (blk + 1) * BS)
                eng.dma_start(xT[:, :, bsl], xT_d[:, :, bsl])

            z2 = const.tile([32, 2, 128], f32)
            nc.vector.memset(z2[:], 0.0)

            # ---- phase A ----
            qk = big.tile([128, L], bf16)    # q 0:64, v' 64:128
            vgp = big.tile([128, L], bf16)   # k' 0:64, gp 64:128
            v0 = big.tile([64, L], bf16)     # v' shifted to base 0
            sg4 = big.tile([64, L], f32)     # rows 0:4 sim/nk, 32:36 glog/nv
            with tc.tile_pool(name="pmm", bufs=2, space="PSUM") as pmm, \
                 tc.tile_pool(name="psmall", bufs=2, space="PSUM") as psmall:
                for blk in range(NBLK):
                    bsl = slice(blk * BS, (blk + 1) * BS)
                    ps1 = pmm.tile([128, BS], f32, tag="mm")
                    for f in range(2):
                        nc.tensor.matmul(ps1[:], w1T[:, f, :], xT[:, f, bsl],
                                         start=(f == 0), stop=(f == 1))
                    nc.scalar.copy(qk[:, bsl], ps1[:])
                    ps2 = pmm.tile([128, BS], f32, tag="mm")
                    for f in range(2):
                        nc.tensor.matmul(ps2[:], w2T[:, f, :], xT[:, f, bsl],
                                         start=(f == 0), stop=(f == 1))
                    nc.vector.tensor_copy(vgp[:, bsl], ps2[:])
                    s1 = work.tile([128, BS], bf16, tag="s1")
                    nc.vector.tensor_mul(s1[0:64, :], qk[0:64, bsl], vgp[0:64, bsl])
                    nc.gpsimd.tensor_mul(s1[64:128, :], vgp[0:64, bsl], vgp[0:64, bsl])
                    s2 = work.tile([128, BS], bf16, tag="s2")
                    nc.gpsimd.tensor_mul(s2[0:64, :], qk[64:128, bsl], qk[64:128, bsl])
                    nc.vector.tensor_mul(s2[64:128, :], qk[64:128, bsl], vgp[64:128, bsl])
                    cs = psmall.tile([64, BS], f32, tag="cs")
                    hpA = tc.high_priority()
                    hpA.__enter__()
                    nc.tensor.matmul(cs[0:32, :], cw1[:, :], s1[:], start=True, stop=True)
                    nc.tensor.matmul(cs[32:64, :], cw2[:, :], s2[:], start=True, stop=True)
                    if blk % 2 == 0:
                        nc.scalar.copy(sg4[:, bsl], cs[:])
                    else:
                        nc.vector.tensor_copy(sg4[:, bsl], cs[:])
                    hpA.__exit__(None, None, None)
                    nc.sync.dma_start(v0[:, bsl], qk[64:128, bsl])

            # ---- phase B ----
            po_cm = tc.tile_pool(name="po", bufs=5, space="PSUM")
            po = po_cm.__enter__()
            pS_cm = tc.tile_pool(name="pS", bufs=1, space="PSUM")
            pS = pS_cm.__enter__()
            py_cm = tc.tile_pool(name="py", bufs=2, space="PSUM")
            py = py_cm.__enter__()
            vR = big.tile([128, NCH, 64], bf16)
            khR = big.tile([128, NCH, 64], bf16)
            # prefill k' transposes + plain drains (scan-independent)
            for c in range(NCH):
                csl = slice(c * C, (c + 1) * C)
                tk = po.tile([128, 64], bf16, tag="o")
                nc.tensor.transpose(tk[:], vgp[0:64, csl], identb[0:64, 0:64])
                if c % 2 == 0:
                    nc.vector.tensor_copy(khR[:, c, :], tk[:])
                else:
                    nc.scalar.copy(khR[:, c, :], tk[:])
            hp_cm = tc.high_priority()
            hp_cm.__enter__()
            sim32 = work.tile([32, 128], f32, tag="sim32")
            nk32 = work.tile([32, 128], f32, tag="nk32")
            glog32 = work.tile([32, 128], f32, tag="glog32")
            nv32 = work.tile([32, 128], f32, tag="nv32")
            for i, (dst, r0) in enumerate(((sim32, 0), (nk32, 2), (glog32, 32), (nv32, 34))):
                nc.sync.dma_start(
                    dst[:],
                    sg4[r0:r0 + 2, :].rearrange("p (s t) -> p s t", s=SEG))
            # e and g stacked in eg [32, 2, 128]
            eg = work.tile([32, 2, 128], f32, tag="eg")
            nc.scalar.activation(eg[:, 0, :], sim32[:], AF.Exp, bias=0.0, scale=qs32)
            rtk = work.tile([32, 128], f32, tag="rtk")
            nc.scalar.sqrt(rtk[:], nk32[:])
            rtv = work.tile([32, 128], f32, tag="rtv")
            nc.scalar.sqrt(rtv[:], nv32[:])
            rik = work.tile([32, 128], f32, tag="rik")
            nc.vector.reciprocal(rik[:], rtk[:])
            riv = work.tile([32, 128], f32, tag="riv")
            nc.vector.reciprocal(riv[:], rtv[:])
            nrm = work.tile([32, 128], f32, tag="nrm")
            nc.vector.tensor_mul(nrm[:], rik[:], riv[:])
            glogs = work.tile([32, 128], f32, tag="glogs")
            nc.vector.tensor_mul(glogs[:], glog32[:], nrm[:])
            grelu = work.tile([32, 128], f32, tag="grelu")
            nc.scalar.activation(grelu[:], glogs[:], AF.Relu, bias=gb, scale=1.0)
            gsq = work.tile([32, 128], f32, tag="gsq")
            nc.scalar.square(gsq[:], grelu[:])
            nc.vector.tensor_scalar_add(eg[:, 1, :], gsq[:], EPS)
            ghat = work.tile([32, 128], bf16, tag="ghat")
            nc.vector.tensor_mul(ghat[:], eg[:, 1, :], nrm[:])
            gR_ps = py.tile([128, 32], bf16, tag="y")
            nc.tensor.transpose(gR_ps[:], ghat[:], identb[0:32, 0:32])
            gR = work.tile([128, 32], f32, tag="gR")
            nc.vector.tensor_copy(gR[:], gR_ps[:])
            # cumsums (separate scans, shared dest)
            ecs = work.tile([32, 2, 128], f32, tag="ecs")
            nc.vector.tensor_tensor_scan(ecs[:, 0, :], eg[:, 0, :], z2[:, 0, :],
                                         0.0, OP.add, OP.add)
            nc.vector.tensor_tensor_scan(ecs[:, 1, :], eg[:, 1, :], z2[:, 0, :],
                                         0.0, OP.add, OP.add)
            offs_ps = py.tile([32, 2], f32, tag="y")
            nc.tensor.matmul(offs_ps[:], lt32[:, :], ecs[:, :, 127:128], start=True, stop=True)
            offs = work.tile([32, 2], f32, tag="offs")
            nc.vector.tensor_scalar_add(offs[:], offs_ps[:], EPS)
            sgs = work.tile([32, 2, 128], f32, tag="sgs")
            nc.vector.tensor_scalar(sgs[:, 0, :], ecs[:, 0, :], offs[:, 0:1], None, OP.add)
            nc.scalar.activation(sgs[:, 1, :], ecs[:, 1, :], AF.Identity,
                                 bias=offs[:, 1:2], scale=1.0)
            rsg = work.tile([32, 2, 128], f32, tag="rsg")
            nc.vector.reciprocal(rsg[:], sgs[:])
            sw32 = work.tile([32, 128], f32, tag="sw32")
            nc.vector.tensor_mul(sw32[:], eg[:, 0, :], rsg[:, 0, :])
            coef = work.tile([32, 128], f32, tag="coef")
            if DEBUG:
                sig = work.tile([32, 128], f32, tag="sig")
                nc.scalar.activation(sig[:], sw32[:], AF.Sigmoid, bias=0.0, scale=1.0)
                nc.vector.tensor_mul(coef[:], sw32[:], sig[:])
            else:
                nc.scalar.activation(coef[:], sw32[:], AF.Silu, bias=0.0, scale=1.0)
            w32 = work.tile([32, 128], bf16, tag="w32")
            nc.vector.scalar_tensor_tensor(w32[:], coef[:], 1.0, rsg[:, 1, :],
                                           OP.add, OP.mult)
            # w broadcast -> qw = q * w
            w2 = work.tile([2, L], bf16, tag="w2")
            nc.sync.dma_start(w2.rearrange("p (s t) -> p s t", s=SEG), w32[:])
            qw = big.tile([64, L], bf16)
            qw_cuts = [0, 128, 512, 1024, 1536, 2048]
            for bi in range(len(qw_cuts) - 1):
                bsl = slice(qw_cuts[bi], qw_cuts[bi + 1])
                wbc = py.tile([64, BS], f32, tag="y")
                wn = qw_cuts[bi + 1] - qw_cuts[bi]
                nc.tensor.matmul(wbc[:, 0:wn], ones2[:, :], w2[:, bsl],
                                 start=True, stop=True)
                nc.vector.tensor_mul(qw[:, bsl], qk[0:64, bsl], wbc[:, 0:wn])
            hp_cm.__exit__(None, None, None)
            # prefill v' transposes; drains carry ghat scale
            for c in range(NCH):
                csl = slice(c * C, (c + 1) * C)
                tv = po.tile([128, 64], bf16, tag="o")
                nc.tensor.transpose(tv[:], v0[:, csl], identb[0:64, 0:64])
                if c % 2 == 0:
                    nc.vector.tensor_scalar(vR[:, c, 0:32], tv[:, 0:32],
                                            gR[:, c:c + 1], None, OP.mult)
                else:
                    nc.scalar.activation(vR[:, c, 0:32], tv[:, 0:32], AF.Copy,
                                         bias=0.0, scale=gR[:, c:c + 1])
                nc.vector.tensor_scalar(vR[:, c, 32:64], tv[:, 32:64],
                                        gR[:, 16 + c:16 + c + 1], None, OP.mult)

            # ---- phase C ----
            if True:

                Sf_prev = None
                # S-recurrence first: qw-independent, overlaps scan/w-bcast
                Sall = big.tile([64, NCH, 64], bf16)
                for cidx in range(NCH - 1):
                    dS = pS.tile([64, 64], f32, tag="s")
                    nc.tensor.matmul(dS[:], vR[:, cidx, :], khR[:, cidx, :],
                                     start=True, stop=True)
                    Sf_new = work.tile([64, 64], f32, tag="Sf")
                    if cidx == 0:
                        nc.vector.tensor_copy(Sf_new[:], dS[:])
                    else:
                        nc.vector.tensor_add(Sf_new[:], Sf_prev[:], dS[:])
                    Sf_prev = Sf_new
                    nc.gpsimd.tensor_copy(Sall[:, cidx, :], Sf_new[:])
                # main loop: fully pipelineable
                for cidx in range(NCH):
                    csl = slice(cidx * C, (cidx + 1) * C)
                    atm = work.tile([128, 2, 128], bf16, tag="atm")
                    for lane in range(2):
                        lsl = slice(lane * 32, lane * 32 + 32)
                        ppl = po.tile([128, 128], f32, tag="o")
                        nc.tensor.matmul(ppl[:], v0[lsl, csl], qw[lsl, csl],
                                         start=True, stop=True)
                        if lane == 0:
                            nc.vector.scalar_tensor_tensor(
                                atm[:, 0, :], ppl[:], gR[:, cidx:cidx + 1],
                                mask2[:, 0, :], OP.mult, OP.mult)
                        else:
                            p1sb = work.tile([128, 128], bf16, tag="p1sb")
                            nc.scalar.activation(
                                p1sb[:], ppl[:], AF.Copy, bias=0.0,
                                scale=gR[:, 16 + cidx:16 + cidx + 1])
                            nc.gpsimd.tensor_mul(atm[:, 1, :], p1sb[:],
                                                 mask2[:, 1, :])
                    # ctT = khR^T atm + S^T qw   [64 (lane,e), 128 j]
                    ctT_ps = po.tile([64, 128], f32, tag="o")
                    for lane in range(2):
                        lsl = slice(lane * 32, lane * 32 + 32)
                        nc.tensor.matmul(ctT_ps[lsl, :], khR[:, cidx, lsl],
                                         atm[:, lane, :],
                                         start=True, stop=(cidx == 0))
                        if cidx > 0:
                            nc.tensor.matmul(
                                ctT_ps[lsl, :],
                                Sall[lsl, cidx - 1, lane * 32:lane * 32 + 32],
                                qw[lsl, csl],
                                start=False, stop=True)
                    ctT = outp.tile([64, 128], bf16, tag="ctT")
                    if cidx % 2 == 0:
                        nc.scalar.copy(ctT[:], ctT_ps[:])
                    else:
                        nc.vector.tensor_copy(ctT[:], ctT_ps[:])
                    # y
                    if cidx % 2 == 0:
                        y_ps = py.tile([128, 2, 256], f32, tag="y")
                    nc.tensor.matmul(y_ps[:, cidx % 2, :], ctT[:], woT[0:64, :],
                                     start=True, stop=True)
                    if cidx % 2 == 1:
                        y_sb = outp.tile([128, 2, 256], bf16, tag="ysb")
                        nc.scalar.copy(y_sb[:, 0, :], y_ps[:, 0, :])
                        nc.vector.tensor_copy(y_sb[:, 1, :], y_ps[:, 1, :])
                        nc.sync.dma_start(
                            y_d[(cidx - 1) * C:(cidx + 1) * C, :].rearrange(
                                "(a p) d -> p a d", a=2),
                            y_sb[:])
                if DEBUG:
                    nc.sync.dma_start(dqk_d[:, :], qk[:])
                    nc.sync.dma_start(dvgp_d[:, :], vgp[:])
                    dsc = const.tile([32, 128, 6], f32, tag="dsc")
                    nc.vector.tensor_copy(dsc[:, :, 0], sim32[:])
                    nc.vector.tensor_copy(dsc[:, :, 1], nk32[:])
                    nc.vector.tensor_copy(dsc[:, :, 2], nv32[:])
                    nc.vector.tensor_copy(dsc[:, :, 3], glog32[:])
                    nc.vector.tensor_copy(dsc[:, :, 4], eg[:, 1, :])
                    nc.vector.tensor_copy(dsc[:, :, 5], w32[:])
                    nc.sync.dma_start(dsc_d[:, :, :], dsc[:])
                    nc.sync.dma_start(dkh_d[:, :], khR[:].rearrange("p a b -> p (a b)"))
                    nc.sync.dma_start(dqw_d[:, :], qw[:])
            py_cm.__exit__(None, None, None)
            pS_cm.__exit__(None, None, None)
            po_cm.__exit__(None, None, None)

    global _LAST_TC_SPAN
    try:
        _LAST_TC_SPAN = max(e[2] for e in tc._perfetto_entries if e[2] is not None)
    except Exception:
        _LAST_TC_SPAN = 0
    nc.compile()
    return nc


_NC_CACHE = None
_LAST_IN_MAPS = None
_LAST_TC_SPAN = 0


def _get_nc():
    global _NC_CACHE
    if _NC_CACHE is None:
        _NC_CACHE = _build_bass()
    return _NC_CACHE


def _bf16(a):
    import ml_dtypes
    return np.asarray(a, dtype=np.float32).astype(ml_dtypes.bfloat16)


def kernel(**inputs):
    import sys
    if '/opt/trn_rl_repo' not in sys.path:
        sys.path.insert(0, '/opt/trn_rl_repo')
    from concourse.bass_utils import run_bass_kernel_spmd

    inp = {k: np.asarray(v) for k, v in inputs.items()}
    x = inp['x'].astype(np.float32)
    wq, wk, wv, wo = (inp[n].astype(np.float32) for n in ('wq_w', 'wk_w', 'wv_w', 'wo_w'))
    wg = inp['wg_w'].astype(np.float32).reshape(HD, HD)
    gbv = float(inp['wg_b'][0])
    kvs = inp['kv_norm_scale'].astype(np.float32)[0, :, 0]
    qks = inp['qk_norm_scale'].astype(np.float32)[0, :, 0]

    nc = _get_nc()

    identb = np.eye(128, dtype=np.float32)
    maskc = (np.arange(128)[:, None] <= np.arange(128)[None, :]).astype(np.float32)
    lt32 = np.zeros((32, 32), np.float32)
    for p in range(32):
        for s in range(32):
            if p // 16 == s // 16 and s % 16 < p % 16:
                lt32[s, p] = 1.0

    in_maps = []
    for core in range(N_CORES):
        b = core // 4
        heads = (2 * (core % 4), 2 * (core % 4) + 1)
        xT = np.ascontiguousarray(
            x[b].T.reshape(2, 128, L).transpose(1, 0, 2))  # [128,2,L]

        a_v = np.empty((2, HD), np.float32)
        b_v = np.empty((2, HD), np.float32)
        mg = []
        for li, hh in enumerate(heads):
            sc = kvs[hh]
            a_v[li] = sc[:, 0]
            b_v[li] = sc[0, :] / sc[0, 0]
            mg.append(wg * sc)

        # W1 = {q, v'}; W2 = {k', gp}
        W1 = np.empty((128, D), np.float32)
        W2 = np.empty((128, D), np.float32)
        for li, hh in enumerate(heads):
            W1[li * 32:li * 32 + 32] = wq[hh * HD:(hh + 1) * HD, :]
            W1[64 + li * 32:64 + li * 32 + 32] = a_v[li][:, None] * wv[hh * HD:(hh + 1) * HD, :]
            W2[li * 32:li * 32 + 32] = b_v[li][:, None] * wk[hh * HD:(hh + 1) * HD, :]
            W2[64 + li * 32:64 + li * 32 + 32] = (
                (1.0 / a_v[li])[:, None] * (mg[li] @ wk[hh * HD:(hh + 1) * HD, :]))
        w1T = np.ascontiguousarray(W1.T.reshape(2, 128, 128).transpose(1, 0, 2))
        w2T = np.ascontiguousarray(W2.T.reshape(2, 128, 128).transpose(1, 0, 2))

        cw1 = np.zeros((128, 32), np.float32)
        cw2 = np.zeros((128, 32), np.float32)
        for li in range(2):
            cw1[li * 32:(li + 1) * 32, li] = 1.0 / b_v[li]                   # sim
            cw1[64 + li * 32:64 + (li + 1) * 32, 2 + li] = 1.0 / b_v[li] ** 2  # |k|^2
            cw2[64 + li * 32:64 + (li + 1) * 32, li] = 1.0                   # glog
            cw2[li * 32:(li + 1) * 32, 2 + li] = 1.0 / a_v[li] ** 2          # |v|^2

        woT = np.empty((64, D), np.float32)
        for li, hh in enumerate(heads):
            woT[li * 32:(li + 1) * 32, :] = wo[:, hh * HD:(hh + 1) * HD].T

        ones2 = np.zeros((2, 64), np.float32)
        ones2[0, 0:32] = 1.0
        ones2[1, 32:64] = 1.0

        wb = np.zeros((128, 1280), np.float32)
        wb[:, 0:256] = w1T.reshape(128, 256)
        wb[:, 256:512] = w2T.reshape(128, 256)
        wb[:, 512:544] = cw1
        wb[:, 544:576] = cw2
        wb[0:64, 576:832] = woT
        wb[:, 832:960] = identb
        wb[:, 960:1088] = maskc
        wb[:, 1088:1216] = maskc
        wb[0:2, 1216:1280] = ones2

        sbm = np.zeros((32, 34), np.float32)
        sbm[0:16, 0] = qks[heads[0]]
        sbm[16:32, 0] = qks[heads[1]]
        sbm[:, 1] = gbv
        sbm[:, 2:34] = lt32

        in_maps.append({"xT": _bf16(xT), "wb": _bf16(wb), "sb": sbm})

    global _LAST_IN_MAPS
    _LAST_IN_MAPS = in_maps
    res = run_bass_kernel_spmd(nc, in_maps, core_ids=list(range(N_CORES)))
    out = np.zeros((B, L, D), np.float32)
    for core in range(N_CORES):
        out[core // 4] += np.asarray(res.results[core]["y"], dtype=np.float32)
    out += inp['wo_b'].astype(np.float32)[None, None, :]
    return out
